# revision 48
# baseline (speedup 1.0000x reference)
"""GCN (3-layer + mean-pool + FC) on 8 Trainium2 NeuronCores via Bass.

Self-contained: host-side numpy preprocessing shards nodes (at graph
boundaries) and edges (by destination) across 8 cores, builds one SPMD
Bass program, runs it via run_bass_kernel_spmd, and reassembles the
full [512, 11] output.

Algorithm per GCN layer (h' := dis * (g @ W), dis := rsqrt(deg+1)):
  agg[d] = dis_d * ( sum_{e: dst=d} ew_e * h'[src_e]  +  h'[d] ) + b
computed as dense selection matmuls.  Edges are sorted by destination,
cut into 128-slot chunks whose destinations fall inside one 128-wide
window of a 512-dst block; per chunk PSUM accumulates
  S_T[f, dw] += gathered[slot, f]^T @ Msel[slot, dw]
with Msel[slot, dw] = (iota128 == dstw[slot]) * ew[slot] built in one
DVE op, and the self-loop added via identity matmuls of the local h'
block (which also initialize the PSUM).

The per-edge source-row gather uses the SWDGE dma_gather instruction
(~0.34ns/row descriptor gen vs ~10ns/row for indirect DMA): one gather
per (dst-block, table-quarter), where the allgathered bf16 h' table
(8*N_LOC rows) is addressed in 2*N_LOC-row quarters so row indices fit
in int16.  Chunks are quarter-pure; chunk structure is uniform across
cores (max over cores per cell) so one SPMD program serves all 8.
"""
import sys
import numpy as np

sys.path.insert(0, '/opt/trn_rl_repo')

N_CORES = 8
F = 128          # feature dim (in = hid = 128)
OUT_DIM = 11
BLK = 512        # dst nodes per dst-block (psum + epilogue granularity)
WIN = 128        # dst window per chunk matmul
WPB = BLK // WIN


def _preprocess(x, edge_index, batch, edge_weight, n_graphs):
    """Shard nodes/edges across cores; build per-core device arrays and the
    (core-uniform) chunk structure."""
    import ml_dtypes
    n_nodes = x.shape[0]
    batch = np.asarray(batch).astype(np.int64)
    src = np.asarray(edge_index[0]).astype(np.int64)
    dst = np.asarray(edge_index[1]).astype(np.int64)
    ew = np.asarray(edge_weight).astype(np.float32)

    # --- node shards cut at graph boundaries ---
    gstart = np.searchsorted(batch, np.arange(n_graphs + 1))
    cuts = [0]
    for c in range(1, N_CORES):
        target = c * n_nodes / N_CORES
        g = int(np.searchsorted(gstart, target))
        if g > 0 and target - gstart[g - 1] < gstart[g] - target:
            g = g - 1
        g = min(max(g, cuts[-1]), n_graphs)
        cuts.append(g)
    cuts.append(n_graphs)
    cuts = np.array(cuts)
    node_lo = gstart[cuts[:-1]]
    node_hi = gstart[cuts[1:]]
    n_loc_real = node_hi - node_lo
    N_LOC = int(np.ceil(max(n_loc_real.max(), 1) / BLK) * BLK)
    assert 2 * N_LOC <= 32767, "table quarter must fit int16 indices"
    NBLK = N_LOC // BLK
    NCOL = N_LOC // 128
    QS = 2 * N_LOC           # rows per table quarter
    n_graphs_core = cuts[1:] - cuts[:-1]
    G_LOC = int(n_graphs_core.max())

    owner = np.searchsorted(node_hi, np.arange(n_nodes), side='right')
    local = np.arange(n_nodes) - node_lo[owner]
    # quarter table: node (c, r) lives in quarter r//QN at row c*QN + r%QN
    QN = N_LOC // 4
    q_of = local // QN
    qrow = owner * QN + local % QN      # row within its quarter table

    # --- edges assigned by dst; cells = (block, src-quarter, dst-window) ---
    e_core = owner[dst]
    e_dstl = local[dst]
    e_q = q_of[src]
    e_b = e_dstl // BLK
    NCELL = NBLK * 4
    e_key = e_b * 4 + e_q

    cnt = np.zeros((N_CORES, NCELL), np.int64)
    for c in range(N_CORES):
        np.add.at(cnt[c], e_key[e_core == c], 1)
    nch_cell = np.ceil(cnt.max(axis=0) / 128).astype(np.int64)  # [NCELL]
    cell_off = np.concatenate([[0], np.cumsum(nch_cell)])
    NCH = int(cell_off[-1])

    # program structure (python constants, identical for all cores).
    # Chunks are 128 dst-sorted edges of one (block, quarter) cell; a chunk's
    # dsts may straddle window boundaries, so each chunk carries the set of
    # windows it may touch (max over cores, padded chunks touch window 0).
    chunk_wins = [set() for _ in range(NCH)]
    blocks = []
    for b in range(NBLK):
        groups = []
        for q in range(4):
            cell = b * 4 + q
            c0 = int(cell_off[cell])
            nchg = int(cell_off[cell + 1] - c0)
            if nchg > 0:
                groups.append((q, c0, nchg))
        c0b = int(cell_off[b * 4])
        chb = int(cell_off[(b + 1) * 4]) - c0b
        blocks.append(dict(groups=groups, c0=c0b, chb=chb))
    MAXCHB = max(bl['chb'] for bl in blocks) if NCH else 1

    # --- per-core device arrays ---
    dstw = np.zeros((N_CORES, 128, max(NCH, 1)), np.float32)
    ewa = np.zeros((N_CORES, 128, max(NCH, 1)), np.float32)
    idx16 = np.zeros((N_CORES, 128, max(NCH, 1) * 8), np.int16)

    deg_cnt = np.zeros((N_CORES, N_LOC), np.int64)
    for c in range(N_CORES):
        m = e_core == c
        np.add.at(deg_cnt[c], e_dstl[m], 1)
    ELLW = max(int(deg_cnt.max()), 1)
    ell = np.zeros((N_CORES, 128, NCOL * ELLW), np.float32)

    for c in range(N_CORES):
        m = np.where(e_core == c)[0]
        k = e_key[m]
        order = np.lexsort((e_dstl[m], k))   # cell-major, dst minor
        me = m[order]
        ks = k[order]
        cell_start = np.searchsorted(ks, np.arange(NCELL))
        pos = np.arange(len(me)) - cell_start[ks]
        slot = cell_off[ks] * 128 + pos
        cid = slot // 128
        p = slot % 128
        dstw[c, p, cid] = (e_dstl[me] % BLK).astype(np.float32) - 256.0
        ewa[c, p, cid] = ew[me]
        idxv = qrow[src[me]].astype(np.int16)
        col = cid * 8 + p // 16
        for r in range(8):
            idx16[c, 16 * r + p % 16, col] = idxv
        for cw in set(zip(cid.tolist(), ((e_dstl[me] % BLK) // WIN).tolist())):
            chunk_wins[cw[0]].add(cw[1])
        # ELL by dst for the degree computation
        dl = e_dstl[me]
        order2 = np.argsort(dl, kind='stable')
        dl2 = dl[order2]
        me2 = me[order2]
        node_start = np.searchsorted(dl2, np.arange(N_LOC))
        pos_in_node = np.arange(len(me2)) - node_start[dl2]
        ell[c, dl2 % 128, (dl2 // 128) * ELLW + pos_in_node] = ew[me2]

    gid = np.full((N_CORES, 128, NCOL), -1.0, np.float32)
    cntn = np.ones((N_CORES, 128, NCOL), np.float32)
    gcnt = np.bincount(batch, minlength=n_graphs).astype(np.float32)
    for c in range(N_CORES):
        n = n_loc_real[c]
        ids = np.arange(node_lo[c], node_hi[c])
        rel = batch[ids] - cuts[c]
        li = np.arange(n)
        gid[c, li % 128, li // 128] = rel.astype(np.float32)
        cntn[c, li % 128, li // 128] = gcnt[batch[ids]]

    xT = np.zeros((N_CORES, 128, N_LOC), ml_dtypes.bfloat16)
    for c in range(N_CORES):
        n = n_loc_real[c]
        xT[c, :, :n] = np.asarray(x[node_lo[c]:node_hi[c]]).astype(np.float32).T

    meta = dict(N_LOC=N_LOC, NBLK=NBLK, NCOL=NCOL, NCH=max(NCH, 1), QS=QS,
                blocks=blocks, MAXCHB=MAXCHB, ELLW=ELLW, G_LOC=G_LOC,
                chunk_wins=[sorted(s) for s in chunk_wins],
                n_graphs_core=n_graphs_core.tolist())
    arrays = dict(xT=xT, dstw=dstw, ew=ewa, idx16=idx16, ell=ell,
                  gid=gid, cntn=cntn)
    return meta, arrays


def _build_program(meta, dbg=False):
    from concourse import bass, bacc, tile, mybir

    N_LOC, NBLK, NCH = meta['N_LOC'], meta['NBLK'], meta['NCH']
    NCOL, QS = meta['NCOL'], meta['QS']
    blocks, MAXCHB = meta['blocks'], meta['MAXCHB']
    ELLW, G_LOC = meta['ELLW'], meta['G_LOC']

    nc = bacc.Bacc("TRN2", target_bir_lowering=False, debug=False,
                   num_devices=N_CORES, num_swdge_queues=4)
    f32, bf16, i16 = mybir.dt.float32, mybir.dt.bfloat16, mybir.dt.int16
    AF = mybir.ActivationFunctionType
    OP = mybir.AluOpType

    xT_in = nc.dram_tensor("xT", [128, N_LOC], bf16, kind="ExternalInput")
    dstw_in = nc.dram_tensor("dstw", [128, NCH], f32, kind="ExternalInput")
    ew_in = nc.dram_tensor("ew", [128, NCH], f32, kind="ExternalInput")
    idx_in = nc.dram_tensor("idx16", [128, NCH * 8], i16, kind="ExternalInput")
    ell_in = nc.dram_tensor("ell", [128, NCOL * ELLW], f32, kind="ExternalInput")
    gid_in = nc.dram_tensor("gid", [128, NCOL], f32, kind="ExternalInput")
    cntn_in = nc.dram_tensor("cntn", [128, NCOL], f32, kind="ExternalInput")
    iota_in = nc.dram_tensor("iota128", [128, BLK], bf16, kind="ExternalInput")
    iotaG_in = nc.dram_tensor("iotaG", [128, G_LOC], f32, kind="ExternalInput")
    eye_in = nc.dram_tensor("eye", [128, 128], f32, kind="ExternalInput")
    W_in = [nc.dram_tensor(f"W{l}", [128, 128], bf16, kind="ExternalInput") for l in (1, 2, 3)]
    b12_in = [nc.dram_tensor(f"b{l}", [128, 1], f32, kind="ExternalInput") for l in (1, 2, 3)]
    fcw_in = nc.dram_tensor("fcw", [128, OUT_DIM], f32, kind="ExternalInput")
    fcb_in = nc.dram_tensor("fcbrep", [128, OUT_DIM], f32, kind="ExternalInput")
    y_out = nc.dram_tensor("y", [G_LOC, OUT_DIM], f32, kind="ExternalOutput")
    if dbg:
        dbg_h = nc.dram_tensor("dbg_h", [128, N_LOC], bf16, kind="ExternalOutput")
        dbg_g = nc.dram_tensor("dbg_g", [128, N_LOC], bf16, kind="ExternalOutput")
        dbg_gat = nc.dram_tensor("dbg_gat", [128, meta['MAXCHB'] * 128], bf16,
                                 kind="ExternalOutput")
        dbg_sp = nc.dram_tensor("dbg_sp", [128, BLK], f32, kind="ExternalOutput")

    with tile.TileContext(nc) as tc:
        with (
            tc.tile_pool(name="const", bufs=1) as cpool,
            tc.tile_pool(name="big", bufs=1) as bigpool,
            tc.tile_pool(name="gat", bufs=4) as gatpool,
            tc.tile_pool(name="msel", bufs=24) as mselpool,
            tc.tile_pool(name="work", bufs=2) as workpool,
            tc.tile_pool(name="hp", bufs=2, space="PSUM") as hpsum,
            tc.tile_pool(name="sp", bufs=3, space="PSUM") as spsum,
            tc.tile_pool(name="pp", bufs=1, space="PSUM") as ppsum,
            tc.tile_pool(name="dram", bufs=1, space="DRAM") as dpool,
        ):
            def load(shape, src, tag, dt=f32, pool=cpool):
                t = pool.tile(shape, dt, tag=tag)
                nc.sync.dma_start(t[:], src[:])
                return t
            dstw_t = load([128, NCH], dstw_in, "dstw")
            ew_t = load([128, NCH], ew_in, "ew")
            idx_t = load([128, NCH * 8], idx_in, "idx16", i16)
            gid_t = load([128, NCOL], gid_in, "gid")
            cntn_t = load([128, NCOL], cntn_in, "cntn")
            iota_t = load([128, BLK], iota_in, "iota", bf16)
            iotaG_t = load([128, G_LOC], iotaG_in, "iotaG")
            eye_t = load([128, 128], eye_in, "eye")
            W_t = [load([128, 128], w, f"W{i}", bf16) for i, w in enumerate(W_in)]
            b12_t = [load([128, 1], b, f"b{i}") for i, b in enumerate(b12_in)]
            fcw_t = load([128, OUT_DIM], fcw_in, "fcw")
            fcb_t = load([128, OUT_DIM], fcb_in, "fcb")

            eye_bf = cpool.tile([128, 128], bf16, tag="eyebf")
            nc.vector.tensor_copy(eye_bf[:], eye_t[:])

            gbuf = bigpool.tile([128, N_LOC], bf16, tag="gbuf")
            hloc = bigpool.tile([128, N_LOC], bf16, tag="hloc")
            nc.sync.dma_start(gbuf[:], xT_in[:])

            # ---- degree / dis ----
            ell_t = bigpool.tile([128, NCOL * ELLW], f32, tag="ell")
            nc.sync.dma_start(ell_t[:], ell_in[:])
            deg_t = cpool.tile([128, NCOL], f32, tag="deg")
            nc.vector.tensor_reduce(
                deg_t[:], ell_t[:].rearrange("p (b w) -> p b w", w=ELLW),
                mybir.AxisListType.X, OP.add)
            sq_t = cpool.tile([128, NCOL], f32, tag="sq")
            nc.scalar.activation(sq_t[:], deg_t[:], AF.Sqrt, bias=1.0)
            dis_t = cpool.tile([128, NCOL], f32, tag="dis")
            nc.vector.reciprocal(dis_t[:], sq_t[:])
            invn_t = cpool.tile([128, NCOL], f32, tag="invn")
            nc.vector.reciprocal(invn_t[:], cntn_t[:])

            disT_ps = spsum.tile([NCOL, 128], f32, tag="sp")
            nc.tensor.transpose(disT_ps[:], dis_t[:], eye_t[:])
            disT_sb = cpool.tile([NCOL, 128], f32, tag="disTsb")
            nc.vector.tensor_copy(disT_sb[:], disT_ps[:])
            disdram = dpool.tile([NCOL, 128], f32, tag="disdram")
            nc.sync.dma_start(disdram[:], disT_sb[:])

            # ---- layers ----
            QN = N_LOC // 4              # nodes per quarter table slice
            QCOL = QN // 128
            qrr = [0]                    # gather queue round-robin counter
            for li in range(3):
                Wl = W_t[li]
                ltabQ = [dpool.tile([QN, 128], bf16, tag=f"ltab{li}q{j}",
                                    name=f"ltab{li}q{j}")
                         for j in range(4)]
                tableQ = [dpool.tile([N_CORES * QN, 128], bf16,
                                     tag=f"table{li}q{j}", name=f"table{li}q{j}")
                          for j in range(4)]

                # phase A: h' = dis * (g @ W), node-major in hloc + DRAM tables
                for i in range(NCOL):
                    hp = hpsum.tile([128, 128], f32, tag="hp")
                    nc.tensor.matmul(hp[:], lhsT=gbuf[:, i * 128:(i + 1) * 128],
                                     rhs=Wl[:], start=True, stop=True)
                    nc.scalar.activation(
                        hloc[:, i * 128:(i + 1) * 128], hp[:], AF.Copy,
                        scale=dis_t[:, i:i + 1])
                    j, ji = i // QCOL, i % QCOL
                    nc.sync.dma_start(ltabQ[j][ji * 128:(ji + 1) * 128, :],
                                      hloc[:, i * 128:(i + 1) * 128])

                for j in range(4):
                    nc.gpsimd.collective_compute(
                        "AllGather", OP.bypass,
                        replica_groups=[list(range(N_CORES))],
                        ins=[ltabQ[j].opt()], outs=[tableQ[j].opt()],
                    )

                # aggregation per 512-dst block; epilogue emitted one block
                # late so it never heads the Vector queue before the next
                # block's Msel builds
                pending_epi = []

                def flush_epi():
                    for fn in pending_epi:
                        fn()
                    pending_epi.clear()

                for b in range(NBLK):
                    bl = blocks[b]
                    chb = bl['chb']
                    gat = gatpool.tile([128, MAXCHB * 128], bf16, tag="gat")
                    for (q, c0, nchg) in bl['groups']:
                        for s0 in range(0, nchg, 8):   # <=1024 idx per gather
                            sn = min(8, nchg - s0)
                            c = c0 + s0
                            rel = c - bl['c0']
                            out_ap = gat[:, rel * 128:(rel + sn) * 128].rearrange(
                                "p (c f) -> p c f", f=128)
                            nc.gpsimd.dma_gather(
                                out_ap, tableQ[q][:],
                                idx_t[:, c * 8:(c + sn) * 8],
                                sn * 128, sn * 128, 128,
                                queue_num=qrr[0] % 4)
                            qrr[0] += 1

                    if dbg and li == 0 and b == 0:
                        nc.sync.dma_start(dbg_gat[:], gat[:])

                    sp = spsum.tile([128, BLK], f32, tag="sp")
                    # window-contiguous emission: PSUM accumulation groups
                    # must not interleave regions within a bank
                    win_emits = [[] for _ in range(WPB)]
                    for cid in range(bl['c0'], bl['c0'] + bl['chb']):
                        for w in meta['chunk_wins'][cid]:
                            win_emits[w].append(cid)
                    for j in range(WPB):
                        hblk = hloc[:, (b * WPB + j) * 128:(b * WPB + j + 1) * 128]
                        wc = win_emits[j]
                        nc.tensor.matmul(sp[:, j * 128:(j + 1) * 128],
                                         lhsT=hblk, rhs=eye_bf[:],
                                         start=True, stop=(len(wc) == 0),
                                         skip_group_check=True)
                        for n, cid in enumerate(wc):
                            ms = mselpool.tile([128, WIN], bf16, tag="msel")
                            nc.vector.tensor_scalar(
                                out=ms[:], in0=iota_t[:, j * WIN:(j + 1) * WIN],
                                scalar1=dstw_t[:, cid:cid + 1],
                                scalar2=ew_t[:, cid:cid + 1],
                                op0=OP.is_equal, op1=OP.mult)
                            r = cid - bl['c0']
                            nc.tensor.matmul(sp[:, j * WIN:(j + 1) * WIN],
                                             lhsT=gat[:, r * 128:(r + 1) * 128],
                                             rhs=ms[:],
                                             start=False, stop=(n == len(wc) - 1),
                                             skip_group_check=True)
                    if dbg and li == 0 and b == 0:
                        spf = workpool.tile([128, BLK], f32, tag="spf")
                        nc.vector.tensor_copy(spf[:], sp[:])
                        nc.sync.dma_start(dbg_sp[:], spf[:])

                    def make_epi(b=b, sp=sp):
                        gslice = gbuf[:, b * BLK:(b + 1) * BLK]
                        dr = workpool.tile([128, WPB, 128], f32, tag="disrep")
                        nc.sync.dma_start(
                            dr[:], disdram[b * WPB:(b + 1) * WPB, :]
                            .partition_broadcast(128))
                        t1 = workpool.tile([128, BLK], f32, tag="t1")
                        nc.vector.tensor_tensor(
                            out=t1[:], in0=sp[:],
                            in1=dr[:].rearrange("p a b -> p (a b)"), op=OP.mult)
                        nc.scalar.activation(gslice, t1[:], AF.Relu,
                                             bias=b12_t[li][:])
                    flush_epi()
                    pending_epi.append(make_epi)
                flush_epi()
                if dbg and li == 0:
                    nc.sync.dma_start(dbg_h[:], hloc[:])
                    nc.sync.dma_start(dbg_g[:], gbuf[:])

            # ---- pooling (gbuf holds g3 node-feature-major; transpose per column) ----
            pp = ppsum.tile([128, G_LOC], f32, tag="pp")
            for i in range(NCOL):
                g3f = workpool.tile([128, 128], f32, tag="g3f")
                nc.vector.tensor_copy(g3f[:], gbuf[:, i * 128:(i + 1) * 128])
                tp = hpsum.tile([128, 128], f32, tag="hp")
                nc.tensor.transpose(tp[:], g3f[:], eye_t[:])
                g3n = workpool.tile([128, 128], bf16, tag="g3n")
                nc.vector.tensor_copy(g3n[:], tp[:])
                P = mselpool.tile([128, G_LOC], bf16, tag="P")
                nc.vector.tensor_scalar(
                    out=P[:], in0=iotaG_t[:], scalar1=gid_t[:, i:i + 1],
                    scalar2=invn_t[:, i:i + 1], op0=OP.is_equal, op1=OP.mult)
                nc.tensor.matmul(pp[:], lhsT=g3n[:], rhs=P[:],
                                 start=(i == 0), stop=(i == NCOL - 1),
                                 skip_group_check=True)
            pooledT = cpool.tile([128, G_LOC], f32, tag="pooledT")
            nc.vector.tensor_copy(pooledT[:], pp[:])

            fp = ppsum.tile([128, OUT_DIM], f32, tag="fc")
            nc.tensor.matmul(fp[:G_LOC, :], lhsT=pooledT[:], rhs=fcw_t[:],
                             start=True, stop=True)
            yt = cpool.tile([128, OUT_DIM], f32, tag="yt")
            nc.vector.tensor_tensor(out=yt[:G_LOC, :], in0=fp[:G_LOC, :],
                                    in1=fcb_t[:G_LOC, :], op=OP.add)
            nc.sync.dma_start(y_out[:], yt[:G_LOC, :])

    nc.compile()
    return nc


def _make_in_maps(meta, arrays, W1, b1, W2, b2, W3, b3, fcW, fcb):
    import ml_dtypes
    G_LOC = meta['G_LOC']
    iota128 = np.broadcast_to(np.arange(BLK, dtype=np.float32) - 256.0,
                              (128, BLK)).astype(ml_dtypes.bfloat16).copy()
    iotaG = np.broadcast_to(np.arange(G_LOC, dtype=np.float32), (128, G_LOC)).copy()
    eye = np.eye(128, dtype=np.float32)
    fcbrep = np.broadcast_to(np.asarray(fcb, np.float32), (128, OUT_DIM)).copy()
    common = {
        "iota128": iota128, "iotaG": iotaG, "eye": eye,
        "W1": np.asarray(W1, np.float32).astype(ml_dtypes.bfloat16),
        "W2": np.asarray(W2, np.float32).astype(ml_dtypes.bfloat16),
        "W3": np.asarray(W3, np.float32).astype(ml_dtypes.bfloat16),
        "b1": np.asarray(b1, np.float32).reshape(128, 1),
        "b2": np.asarray(b2, np.float32).reshape(128, 1),
        "b3": np.asarray(b3, np.float32).reshape(128, 1),
        "fcw": np.asarray(fcW, np.float32),
        "fcbrep": fcbrep,
    }
    in_maps = []
    for c in range(N_CORES):
        m = dict(common)
        for k in ("xT", "dstw", "ew", "idx16", "ell", "gid", "cntn"):
            m[k] = arrays[k][c]
        in_maps.append(m)
    return in_maps


def run(x, edge_index, batch, edge_weight, W1, b1, W2, b2, W3, b3, fcW, fcb,
        n_graphs=512, trace=False):
    from concourse import bass_utils
    meta, arrays = _preprocess(x, edge_index, batch, edge_weight, n_graphs)
    nc = _build_program(meta)
    in_maps = _make_in_maps(meta, arrays, W1, b1, W2, b2, W3, b3, fcW, fcb)
    res = bass_utils.run_bass_kernel_spmd(
        nc, in_maps, core_ids=list(range(N_CORES)), trace=trace)
    ng = meta['n_graphs_core']
    y = np.concatenate([res.results[c]["y"][:ng[c]] for c in range(N_CORES)], axis=0)
    return y.astype(np.float32), res


def kernel(x, edge_index, batch, edge_weight, W1, b1, W2, b2, W3, b3, fcW, fcb):
    y, _ = run(np.asarray(x), np.asarray(edge_index), np.asarray(batch),
               np.asarray(edge_weight), W1, b1, W2, b2, W3, b3, fcW, fcb,
               n_graphs=512, trace=False)
    return y
